# revision 9
# baseline (speedup 1.0000x reference)
"""ConcatCritic pair-grid MLP on 8 TRN2 NeuronCores — fp8 DoubleRow version.

Computes out[i, j] = f(x[i], y[j]) where f is a 3-hidden-layer MLP over the
concatenated pair, decomposed so the first layer is two small projections
summed by broadcast.

Sharding: the B^2 pair grid is split row-wise (x batch) across 8 cores;
y and all MLP parameters are replicated. Each core produces a [B/8, B]
score tile; the host concatenates them. b3 (a scalar) is added on the host.

Numerics: layers 1-2 run on the PE array in fp8e4m3 with DoubleRow perf
mode (two 128-row k-groups per instruction at 0.5 cycles/row). Activations
are stored as single fp8 with power-of-2 scales; W1 is split hi+lo into two
fp8 products to cut its quantization error; W2 is a single fp8 product.
Each layer's bias is folded into one extra DoubleRow matmul against a
constant ones tile (the bias is hi/lo-encoded across 64 lhsT slots), which
makes every PSUM drain a pure scale/relu op. Layer 3 is restructured: the
L2 drains compute h2w = relu(z2)*W3[k]/(s1*t2) per k-block, three bf16
adds fold k, and a single [128,1] ones-matmul does the partition sum, so
the scores land in PSUM already descaled (host adds b3 only).

Pipeline: iteration t issues layer0(t+1) on ACT/Pool, then L1(t), L2(t-1),
L3(t-2) on the PE, so the PE never waits on an engine drain in steady
state. PSUM: L1 uses two 2-bank pair tiles, L2 three single-bank tiles,
scores one dedicated bank (8 banks total).
"""

import numpy as np
import ml_dtypes

import concourse.bass as bass
import concourse.mybir as mybir
from concourse import bacc
from concourse.bass_utils import run_bass_kernel_spmd
from concourse.tile import TileContext

B = 256
A_DIM = 128
HID = 512
N_CORES = 8
ROWS = B // N_CORES  # 32 x-rows per core
KB = HID // 128  # 4 k-blocks of 128
PAIR = 512  # pairs per tile = 2 x-rows x 256 y-rows
ROWS_PER_TILE = PAIR // B  # 2
N_TILES = ROWS // ROWS_PER_TILE  # 16

F32 = mybir.dt.float32
F32R = mybir.dt.float32r
F8 = mybir.dt.float8e4
BF16 = mybir.dt.bfloat16
E4 = ml_dtypes.float8_e4m3
BF = ml_dtypes.bfloat16
DR = mybir.MatmulPerfMode.DoubleRow

_CACHE = {}


def _build_nc(sc1):
    nc = bacc.Bacc()

    xT = nc.declare_dram_parameter("xT", [A_DIM, ROWS], F32R, isOutput=False)
    yT = nc.declare_dram_parameter("yT", [A_DIM, B], F32R, isOutput=False)
    Wx = nc.declare_dram_parameter("Wx", [A_DIM, HID], F32R, isOutput=False)
    Wy = nc.declare_dram_parameter("Wy", [A_DIM, HID], F32R, isOutput=False)
    b0s = nc.declare_dram_parameter("b0s", [128, KB], F32, isOutput=False)
    W1hi = nc.declare_dram_parameter("W1hi", [128, KB, HID], F8, isOutput=False)
    W1lo = nc.declare_dram_parameter("W1lo", [128, KB, HID], F8, isOutput=False)
    W2q = nc.declare_dram_parameter("W2q", [128, KB, HID], F8, isOutput=False)
    w3s = nc.declare_dram_parameter("w3s", [128, KB], F32, isOutput=False)
    onebf = nc.declare_dram_parameter("onebf", [128, 1], BF16, isOutput=False)
    bias1L = nc.declare_dram_parameter("bias1L", [128, 2, HID], F8, isOutput=False)
    bias2L = nc.declare_dram_parameter("bias2L", [128, 2, HID], F8, isOutput=False)
    ones8 = nc.declare_dram_parameter("ones8", [128, 2, PAIR], F8, isOutput=False)
    out = nc.declare_dram_parameter("out", [1, ROWS * B], F32, isOutput=True)

    relu = mybir.ActivationFunctionType.Relu
    copyf = mybir.ActivationFunctionType.Copy
    add = mybir.AluOpType.add
    amax = mybir.AluOpType.max
    mult = mybir.AluOpType.mult

    with TileContext(nc) as tc:
        with (
            tc.tile_pool(name="const", bufs=1) as const,
            tc.tile_pool(name="h0p", bufs=2) as h0p,
            tc.tile_pool(name="h1p", bufs=2) as h1p,
            tc.tile_pool(name="h2p", bufs=2) as h2p,
            tc.tile_pool(name="h2ap", bufs=2) as h2ap,
            tc.tile_pool(name="h2cp", bufs=2) as h2cp,
            tc.tile_pool(name="psA", bufs=2, space="PSUM") as psA,
            tc.tile_pool(name="psC", bufs=3, space="PSUM") as psC,
            tc.tile_pool(name="psD", bufs=1, space="PSUM") as psD,
        ):
            # ---- replicated constants -----------------------------------
            W1hi_sb = const.tile([128, KB, HID], F8)
            W1lo_sb = const.tile([128, KB, HID], F8)
            W2q_sb = const.tile([128, KB, HID], F8)
            w3s_sb = const.tile([128, KB], F32)
            onebf_sb = const.tile([128, 1], BF16)
            bias1_sb = const.tile([128, 2, HID], F8)
            bias2_sb = const.tile([128, 2, HID], F8)
            ones_sb = const.tile([128, 2, PAIR], F8)
            xT_sb = const.tile([A_DIM, ROWS], F32R)
            yT_sb = const.tile([A_DIM, B], F32R)
            Wx_sb = const.tile([A_DIM, HID], F32R)
            Wy_sb = const.tile([A_DIM, HID], F32R)
            b0s_sb = const.tile([128, KB], F32)

            nc.sync.dma_start(xT_sb[:], xT[:, :])
            nc.sync.dma_start(b0s_sb[:], b0s[:, :])
            for m in range(KB):
                sl = slice(m * 128, (m + 1) * 128)
                nc.sync.dma_start(Wx_sb[:, sl], Wx[:, sl])
            nc.sync.dma_start(yT_sb[:], yT[:, :])
            for m in range(KB):
                sl = slice(m * 128, (m + 1) * 128)
                nc.sync.dma_start(Wy_sb[:, sl], Wy[:, sl])
            nc.sync.dma_start(W1hi_sb[:], W1hi[:, :, :])
            nc.sync.dma_start(W1lo_sb[:], W1lo[:, :, :])
            nc.sync.dma_start(ones_sb[:], ones8[:, :, :])
            nc.sync.dma_start(bias1_sb[:], bias1L[:, :, :])
            nc.sync.dma_start(W2q_sb[:], W2q[:, :, :])
            nc.sync.dma_start(bias2_sb[:], bias2L[:, :, :])
            nc.sync.dma_start(w3s_sb[:], w3s[:, :])
            nc.sync.dma_start(onebf_sb[:], onebf[:, :])

            # ---- input projections (f32r, exact) ------------------------
            bxT = const.tile([128, KB, ROWS], F32)
            hyT = const.tile([128, KB, B], BF16)
            for m in range(KB):
                sl = slice(m * 128, (m + 1) * 128)
                ph = psC.tile([128, PAIR], F32, tag="psC", name="ph")[:, :ROWS]
                nc.tensor.matmul(ph, Wx_sb[:, sl], xT_sb[:], start=True, stop=True)
                nc.vector.tensor_scalar(
                    bxT[:, m], ph, b0s_sb[:, m : m + 1], None, add
                )
                ph2 = psC.tile([128, PAIR], F32, tag="psC", name="ph2")[:, :B]
                nc.tensor.matmul(ph2, Wy_sb[:, sl], yT_sb[:], start=True, stop=True)
                nc.scalar.copy(out=hyT[:, m], in_=ph2)

            # ---- pipeline stages ----------------------------------------
            def layer0(t):
                i0 = t * ROWS_PER_TILE
                h0q = h0p.tile([128, KB, PAIR], F8, tag="h0")
                for k in range(KB):
                    for a in range(ROWS_PER_TILE):
                        dst = h0q[:, k, a * B : (a + 1) * B]
                        bxc = bxT[:, k, i0 + a : i0 + a + 1]
                        if k == 3:
                            nc.scalar.activation(
                                dst, hyT[:, k], relu, bias=bxc, scale=1.0
                            )
                        else:
                            nc.gpsimd.tensor_scalar(
                                dst, hyT[:, k], bxc, 0.0, add, amax
                            )
                return h0q

            def layer1(h0q):
                h1q = h1p.tile([128, KB, PAIR], F8, tag="h1")
                for mp in range(2):
                    pt = psA.tile([128, 2, PAIR], F32, tag="psA", name="pt")
                    for h in range(2):
                        msl = slice((2 * mp + h) * 128, (2 * mp + h + 1) * 128)
                        nc.tensor.matmul(
                            pt[:, h], W1hi_sb[:, 0:2, msl], h0q[:, 0:2, :],
                            start=True, stop=False, perf_mode=DR,
                        )
                        nc.tensor.matmul(
                            pt[:, h], W1hi_sb[:, 2:4, msl], h0q[:, 2:4, :],
                            start=False, stop=False, perf_mode=DR,
                        )
                        nc.tensor.matmul(
                            pt[:, h], W1lo_sb[:, 0:2, msl], h0q[:, 0:2, :],
                            start=False, stop=False, perf_mode=DR,
                        )
                        nc.tensor.matmul(
                            pt[:, h], W1lo_sb[:, 2:4, msl], h0q[:, 2:4, :],
                            start=False, stop=False, perf_mode=DR,
                        )
                        nc.tensor.matmul(
                            pt[:, h], bias1_sb[:, :, msl], ones_sb[:],
                            start=False, stop=True, perf_mode=DR,
                        )
                    # pair drain on ACT: h1 = fp8(relu(psum * sc1))
                    nc.scalar.activation(
                        h1q[:, 2 * mp : 2 * mp + 2, :], pt, relu,
                        bias=0.0, scale=sc1,
                    )
                return h1q

            def layer2(h1q):
                # psum m-blocks drained as h2w[p,k,j] = max(psum,0)*w3s[p,k]
                h2w = h2p.tile([128, KB, PAIR], BF16, tag="h2")
                for m in range(KB):
                    msl = slice(m * 128, (m + 1) * 128)
                    pt = psC.tile([128, PAIR], F32, tag="psC", name="pt2")
                    nc.tensor.matmul(
                        pt, W2q_sb[:, 0:2, msl], h1q[:, 0:2, :],
                        start=True, stop=False, perf_mode=DR,
                    )
                    nc.tensor.matmul(
                        pt, W2q_sb[:, 2:4, msl], h1q[:, 2:4, :],
                        start=False, stop=False, perf_mode=DR,
                    )
                    nc.tensor.matmul(
                        pt, bias2_sb[:, :, msl], ones_sb[:],
                        start=False, stop=True, perf_mode=DR,
                    )
                    nc.vector.tensor_scalar(
                        h2w[:, m], pt, 0.0, w3s_sb[:, m : m + 1], amax, mult
                    )
                h2a = h2ap.tile([128, 2, PAIR], BF16, tag="h2a")
                nc.vector.tensor_tensor(h2a[:, 0], h2w[:, 0], h2w[:, 1], add)
                nc.gpsimd.tensor_tensor(h2a[:, 1], h2w[:, 2], h2w[:, 3], add)
                h2acc = h2cp.tile([128, PAIR], BF16, tag="h2c")
                nc.vector.tensor_tensor(h2acc[:], h2a[:, 0], h2a[:, 1], add)
                return h2acc

            def layer3(t, h2acc):
                ps3 = psD.tile([128, PAIR], F32, tag="psD", name="ps3")[:1]
                nc.tensor.matmul(ps3, onebf_sb[:], h2acc[:], start=True, stop=True)
                sc_sb = h2cp.tile([1, PAIR], F32, tag="sc_sb")
                nc.scalar.activation(sc_sb[:], ps3, copyf, bias=0.0, scale=1.0)
                nc.sync.dma_start(out[:, t * PAIR : (t + 1) * PAIR], sc_sb[:])

            # ---- software-pipelined main loop ---------------------------
            h0s = {0: layer0(0)}
            h1s = {}
            h2s = {}
            for t in range(N_TILES):
                if t + 1 < N_TILES:
                    h0s[t + 1] = layer0(t + 1)
                h1s[t] = layer1(h0s.pop(t))
                if t >= 1:
                    h2s[t - 1] = layer2(h1s.pop(t - 1))
                if t >= 2:
                    layer3(t - 2, h2s.pop(t - 2))
            h2s[N_TILES - 1] = layer2(h1s.pop(N_TILES - 1))
            layer3(N_TILES - 2, h2s.pop(N_TILES - 2))
            layer3(N_TILES - 1, h2s.pop(N_TILES - 1))

    nc.compile()
    return nc


def _q8(a):
    return np.asarray(a, np.float32).astype(E4)


def _p2(m, target=112.0):
    return float(2.0 ** np.floor(np.log2(target / m)))


def _prep(inputs):
    """Host-side quantization; returns (scales, per-core input maps)."""
    f = lambda a: np.ascontiguousarray(np.asarray(a), dtype=np.float32)
    x, y = f(inputs["x"]), f(inputs["y"])
    Wx, Wy, b0 = f(inputs["Wx"]), f(inputs["Wy"]), f(inputs["b0"])
    W1, b1 = f(inputs["W1"]), f(inputs["b1"])
    W2, b2 = f(inputs["W2"]), f(inputs["b2"])
    W3 = f(inputs["W3"])

    hx = x @ Wx
    hy = y @ Wy
    h0max = float(np.max(np.max(hx + b0, 0) + np.max(hy, 0)))
    s0 = _p2(h0max)
    # subsampled forward for the h1 range (16 x-rows), 2x margin
    h0s = np.maximum(hx[::16][:, None, :] + hy[None, :, :] + b0, 0)
    h1s = np.maximum(h0s @ W1 + b1, 0)
    s1 = _p2(float(np.max(h1s)) * 2) * 2
    t1 = _p2(float(np.max(np.abs(W1))))
    t2 = _p2(float(np.max(np.abs(W2))))

    def kmajor(W):  # [HID, N] -> [128, KB, N]
        return np.ascontiguousarray(W.reshape(KB, 128, -1).transpose(1, 0, 2))

    W1hi_f = _q8(W1 * t1).astype(np.float32)
    W1hi = _q8(kmajor(W1hi_f))
    W1lo = _q8(kmajor(W1 * t1 - W1hi_f))
    W2q = _q8(kmajor(W2 * t2))
    w3s = (W3[:, 0] / (s1 * t2)).reshape(KB, 128).T.copy()

    def bias_lhsT(b, S):
        bhi = _q8(b * S / 32.0).astype(np.float32)
        blo = _q8((b * S - 32.0 * bhi) / 32.0).astype(np.float32)
        L = np.zeros((128, 2, HID), np.float32)
        L[:32, 0, :] = bhi[None, :]
        L[32:64, 0, :] = blo[None, :]
        return L.astype(E4)

    shared = {
        "yT": (y * s0).T.copy(),
        "Wx": Wx,
        "Wy": Wy,
        "b0s": ((b0 * s0).reshape(KB, 128).T).copy(),
        "W1hi": W1hi,
        "W1lo": W1lo,
        "W2q": W2q,
        "w3s": w3s.astype(np.float32),
        "onebf": np.ones((128, 1), np.float32).astype(BF),
        "bias1L": bias_lhsT(b1, s0 * t1),
        "bias2L": bias_lhsT(b2, s1 * t2),
        "ones8": np.ones((128, 2, PAIR), np.float32).astype(E4),
    }
    in_maps = []
    for c in range(N_CORES):
        im = dict(shared)
        im["xT"] = ((x[c * ROWS : (c + 1) * ROWS] * s0).T).copy()
        in_maps.append(im)
    scales = (float(s1 / (s0 * t1)),)
    return scales, in_maps


def run(trace=False, **inputs):
    scales, in_maps = _prep(inputs)
    if _CACHE.get("scales") != scales:
        _CACHE["nc"] = _build_nc(*scales)
        _CACHE["scales"] = scales
    nc = _CACHE["nc"]
    res = run_bass_kernel_spmd(nc, in_maps, core_ids=list(range(N_CORES)), trace=trace)
    b3 = np.float32(np.asarray(inputs["b3"]).reshape(-1)[0])
    blocks = [r["out"].reshape(ROWS, B) + b3 for r in res.results]
    return np.concatenate(blocks, axis=0).astype(np.float32), res


def _get_nc():
    return _CACHE["nc"]


def kernel(**inputs):
    out, _ = run(trace=False, **inputs)
    return out


# revision 12
# speedup vs baseline: 1.0002x; 1.0002x over previous
"""ConcatCritic pair-grid MLP on 8 TRN2 NeuronCores — fp8 DoubleRow version.

Computes out[i, j] = f(x[i], y[j]) where f is a 3-hidden-layer MLP over the
concatenated pair, decomposed so the first layer is two small projections
summed by broadcast.

Sharding: the B^2 pair grid is split row-wise (x batch) across 8 cores;
y and all MLP parameters are replicated. Each core produces a [B/8, B]
score tile; the host concatenates them. b3 (a scalar) is added on the host.

Numerics: layers 1-2 run on the PE array in fp8e4m3 with DoubleRow perf
mode (two 128-row k-groups per instruction at 0.5 cycles/row). Activations
are stored as single fp8 with power-of-2 scales; W1 is split hi+lo into two
fp8 products to cut its quantization error; W2 is a single fp8 product.
Each layer's bias is folded into one extra DoubleRow matmul against a
constant ones tile (the bias is hi/lo-encoded across 64 lhsT slots), which
makes every PSUM drain a pure scale/relu op. Layer 3 is restructured: the
L2 drains compute h2w = relu(z2)*W3[k]/(s1*t2) per k-block, three bf16
adds fold k, and a single [128,1] ones-matmul does the partition sum, so
the scores land in PSUM already descaled (host adds b3 only).

Pipeline: iteration t issues layer0(t+1) on ACT/Pool, then L1(t), L2(t-1),
L3(t-2) on the PE, so the PE never waits on an engine drain in steady
state. PSUM: L1 uses two 2-bank pair tiles, L2 three single-bank tiles,
scores one dedicated bank (8 banks total).
"""

import numpy as np
import ml_dtypes

import concourse.bass as bass
import concourse.mybir as mybir
from concourse import bacc
from concourse.bass_utils import run_bass_kernel_spmd
from concourse.tile import TileContext

B = 256
A_DIM = 128
HID = 512
N_CORES = 8
ROWS = B // N_CORES  # 32 x-rows per core
KB = HID // 128  # 4 k-blocks of 128
PAIR = 512  # pairs per tile = 2 x-rows x 256 y-rows
ROWS_PER_TILE = PAIR // B  # 2
N_TILES = ROWS // ROWS_PER_TILE  # 16

F32 = mybir.dt.float32
F32R = mybir.dt.float32r
F8 = mybir.dt.float8e4
BF16 = mybir.dt.bfloat16
E4 = ml_dtypes.float8_e4m3
BF = ml_dtypes.bfloat16
DR = mybir.MatmulPerfMode.DoubleRow

_CACHE = {}


def _build_nc(sc1):
    nc = bacc.Bacc()

    xT = nc.declare_dram_parameter("xT", [A_DIM, ROWS], F32R, isOutput=False)
    yT = nc.declare_dram_parameter("yT", [A_DIM, B], F32R, isOutput=False)
    Wx = nc.declare_dram_parameter("Wx", [A_DIM, HID], F32R, isOutput=False)
    Wy = nc.declare_dram_parameter("Wy", [A_DIM, HID], F32R, isOutput=False)
    b0s = nc.declare_dram_parameter("b0s", [128, KB], F32, isOutput=False)
    W1hi = nc.declare_dram_parameter("W1hi", [128, KB, HID], F8, isOutput=False)
    W1lo = nc.declare_dram_parameter("W1lo", [128, KB, HID], F8, isOutput=False)
    W2q = nc.declare_dram_parameter("W2q", [128, KB, HID], F8, isOutput=False)
    w3s = nc.declare_dram_parameter("w3s", [128, KB], F32, isOutput=False)
    onebf = nc.declare_dram_parameter("onebf", [128, 1], BF16, isOutput=False)
    bias1L = nc.declare_dram_parameter("bias1L", [128, 2, HID], F8, isOutput=False)
    bias2L = nc.declare_dram_parameter("bias2L", [128, 2, HID], F8, isOutput=False)
    ones8 = nc.declare_dram_parameter("ones8", [128, 2, PAIR], F8, isOutput=False)
    out = nc.declare_dram_parameter("out", [1, ROWS * B], F32, isOutput=True)

    relu = mybir.ActivationFunctionType.Relu
    copyf = mybir.ActivationFunctionType.Copy
    add = mybir.AluOpType.add
    amax = mybir.AluOpType.max
    mult = mybir.AluOpType.mult

    with TileContext(nc) as tc:
        with (
            tc.tile_pool(name="const", bufs=1) as const,
            tc.tile_pool(name="h0p", bufs=2) as h0p,
            tc.tile_pool(name="h1p", bufs=2) as h1p,
            tc.tile_pool(name="h2p", bufs=2) as h2p,
            tc.tile_pool(name="h2ap", bufs=2) as h2ap,
            tc.tile_pool(name="h2cp", bufs=2) as h2cp,
            tc.tile_pool(name="psA", bufs=2, space="PSUM") as psA,
            tc.tile_pool(name="psC", bufs=3, space="PSUM") as psC,
            tc.tile_pool(name="psD", bufs=1, space="PSUM") as psD,
        ):
            # ---- replicated constants -----------------------------------
            W1hi_sb = const.tile([128, KB, HID], F8)
            W1lo_sb = const.tile([128, KB, HID], F8)
            W2q_sb = const.tile([128, KB, HID], F8)
            w3s_sb = const.tile([128, KB], F32)
            onebf_sb = const.tile([128, 1], BF16)
            bias1_sb = const.tile([128, 2, HID], F8)
            bias2_sb = const.tile([128, 2, HID], F8)
            ones_sb = const.tile([128, 2, PAIR], F8)
            xT_sb = const.tile([A_DIM, ROWS], F32R)
            yT_sb = const.tile([A_DIM, B], F32R)
            Wx_sb = const.tile([A_DIM, HID], F32R)
            Wy_sb = const.tile([A_DIM, HID], F32R)
            b0s_sb = const.tile([128, KB], F32)

            nc.sync.dma_start(xT_sb[:], xT[:, :])
            nc.sync.dma_start(Wx_sb[:], Wx[:, :])
            nc.sync.dma_start(b0s_sb[:], b0s[:, :])
            nc.sync.dma_start(yT_sb[:], yT[:, :])
            nc.sync.dma_start(Wy_sb[:], Wy[:, :])
            nc.sync.dma_start(W1hi_sb[:], W1hi[:, :, :])
            nc.sync.dma_start(W1lo_sb[:], W1lo[:, :, :])
            nc.sync.dma_start(ones_sb[:], ones8[:, :, :])
            nc.sync.dma_start(bias1_sb[:], bias1L[:, :, :])
            nc.sync.dma_start(W2q_sb[:], W2q[:, :, :])
            nc.sync.dma_start(bias2_sb[:], bias2L[:, :, :])
            nc.sync.dma_start(w3s_sb[:], w3s[:, :])
            nc.sync.dma_start(onebf_sb[:], onebf[:, :])

            # ---- input projections (f32r, exact) ------------------------
            bxT = const.tile([128, KB, ROWS], F32)
            hyT = const.tile([128, KB, B], BF16)
            for m in range(KB):
                sl = slice(m * 128, (m + 1) * 128)
                ph = psC.tile([128, PAIR], F32, tag="psC", name="ph")[:, :ROWS]
                nc.tensor.matmul(ph, Wx_sb[:, sl], xT_sb[:], start=True, stop=True)
                nc.vector.tensor_scalar(
                    bxT[:, m], ph, b0s_sb[:, m : m + 1], None, add
                )
                ph2 = psC.tile([128, PAIR], F32, tag="psC", name="ph2")[:, :B]
                nc.tensor.matmul(ph2, Wy_sb[:, sl], yT_sb[:], start=True, stop=True)
                nc.scalar.copy(out=hyT[:, m], in_=ph2)

            # ---- pipeline stages ----------------------------------------
            def layer0(t):
                i0 = t * ROWS_PER_TILE
                h0q = h0p.tile([128, KB, PAIR], F8, tag="h0")
                for k in range(KB):
                    for a in range(ROWS_PER_TILE):
                        dst = h0q[:, k, a * B : (a + 1) * B]
                        bxc = bxT[:, k, i0 + a : i0 + a + 1]
                        if k == 3 and a == 1:
                            nc.scalar.activation(
                                dst, hyT[:, k], relu, bias=bxc, scale=1.0
                            )
                        else:
                            nc.gpsimd.tensor_scalar(
                                dst, hyT[:, k], bxc, 0.0, add, amax
                            )
                return h0q

            def layer1(h0q):
                h1q = h1p.tile([128, KB, PAIR], F8, tag="h1")
                for mp in range(2):
                    pt = psA.tile([128, 2, PAIR], F32, tag="psA", name="pt")
                    for h in range(2):
                        msl = slice((2 * mp + h) * 128, (2 * mp + h + 1) * 128)
                        nc.tensor.matmul(
                            pt[:, h], W1hi_sb[:, 0:2, msl], h0q[:, 0:2, :],
                            start=True, stop=False, perf_mode=DR,
                        )
                        nc.tensor.matmul(
                            pt[:, h], W1hi_sb[:, 2:4, msl], h0q[:, 2:4, :],
                            start=False, stop=False, perf_mode=DR,
                        )
                        nc.tensor.matmul(
                            pt[:, h], W1lo_sb[:, 0:2, msl], h0q[:, 0:2, :],
                            start=False, stop=False, perf_mode=DR,
                        )
                        nc.tensor.matmul(
                            pt[:, h], W1lo_sb[:, 2:4, msl], h0q[:, 2:4, :],
                            start=False, stop=False, perf_mode=DR,
                        )
                        nc.tensor.matmul(
                            pt[:, h], bias1_sb[:, :, msl], ones_sb[:],
                            start=False, stop=True, perf_mode=DR,
                        )
                    # pair drain on ACT: h1 = fp8(relu(psum * sc1))
                    nc.scalar.activation(
                        h1q[:, 2 * mp : 2 * mp + 2, :], pt, relu,
                        bias=0.0, scale=sc1,
                    )
                return h1q

            def layer2(h1q):
                # psum m-blocks drained as h2w[p,k,j] = max(psum,0)*w3s[p,k]
                h2w = h2p.tile([128, KB, PAIR], BF16, tag="h2")
                for m in range(KB):
                    msl = slice(m * 128, (m + 1) * 128)
                    pt = psC.tile([128, PAIR], F32, tag="psC", name="pt2")
                    nc.tensor.matmul(
                        pt, W2q_sb[:, 0:2, msl], h1q[:, 0:2, :],
                        start=True, stop=False, perf_mode=DR,
                    )
                    nc.tensor.matmul(
                        pt, W2q_sb[:, 2:4, msl], h1q[:, 2:4, :],
                        start=False, stop=False, perf_mode=DR,
                    )
                    nc.tensor.matmul(
                        pt, bias2_sb[:, :, msl], ones_sb[:],
                        start=False, stop=True, perf_mode=DR,
                    )
                    nc.vector.tensor_scalar(
                        h2w[:, m], pt, 0.0, w3s_sb[:, m : m + 1], amax, mult
                    )
                h2a = h2ap.tile([128, 2, PAIR], BF16, tag="h2a")
                nc.vector.tensor_tensor(h2a[:, 0], h2w[:, 0], h2w[:, 1], add)
                nc.gpsimd.tensor_tensor(h2a[:, 1], h2w[:, 2], h2w[:, 3], add)
                h2acc = h2cp.tile([128, PAIR], BF16, tag="h2c")
                nc.vector.tensor_tensor(h2acc[:], h2a[:, 0], h2a[:, 1], add)
                return h2acc

            def layer3(t, h2acc, ps3_hold):
                ps3 = psD.tile([128, PAIR], F32, tag="psD", name="ps3")[:1]
                nc.tensor.matmul(ps3, onebf_sb[:], h2acc[:], start=True, stop=True)
                sc_sb = h2cp.tile([1, PAIR], F32, tag="sc_sb")
                nc.scalar.activation(sc_sb[:], ps3, copyf, bias=0.0, scale=1.0)
                nc.sync.dma_start(out[:, t * PAIR : (t + 1) * PAIR], sc_sb[:])

            # ---- software-pipelined main loop ---------------------------
            h0s = {0: layer0(0)}
            h1s = {}
            h2s = {}
            hold = {}
            for t in range(N_TILES):
                if t + 1 < N_TILES:
                    h0s[t + 1] = layer0(t + 1)
                h1s[t] = layer1(h0s.pop(t))
                if t >= 1:
                    h2s[t - 1] = layer2(h1s.pop(t - 1))
                if t >= 2:
                    layer3(t - 2, h2s.pop(t - 2), hold)
            h2s[N_TILES - 1] = layer2(h1s.pop(N_TILES - 1))
            layer3(N_TILES - 2, h2s.pop(N_TILES - 2), hold)
            layer3(N_TILES - 1, h2s.pop(N_TILES - 1), hold)

    nc.compile()
    return nc


def _q8(a):
    return np.asarray(a, np.float32).astype(E4)


def _p2(m, target=112.0):
    return float(2.0 ** np.floor(np.log2(target / m)))


def _prep(inputs):
    """Host-side quantization; returns (scales, per-core input maps)."""
    f = lambda a: np.ascontiguousarray(np.asarray(a), dtype=np.float32)
    x, y = f(inputs["x"]), f(inputs["y"])
    Wx, Wy, b0 = f(inputs["Wx"]), f(inputs["Wy"]), f(inputs["b0"])
    W1, b1 = f(inputs["W1"]), f(inputs["b1"])
    W2, b2 = f(inputs["W2"]), f(inputs["b2"])
    W3 = f(inputs["W3"])

    hx = x @ Wx
    hy = y @ Wy
    h0max = float(np.max(np.max(hx + b0, 0) + np.max(hy, 0)))
    s0 = _p2(h0max)
    # subsampled forward for the h1 range (16 x-rows), 2x margin
    h0s = np.maximum(hx[::16][:, None, :] + hy[None, :, :] + b0, 0)
    h1s = np.maximum(h0s @ W1 + b1, 0)
    s1 = _p2(float(np.max(h1s)) * 2) * 2
    t1 = _p2(float(np.max(np.abs(W1))))
    t2 = _p2(float(np.max(np.abs(W2))))

    def kmajor(W):  # [HID, N] -> [128, KB, N]
        return np.ascontiguousarray(W.reshape(KB, 128, -1).transpose(1, 0, 2))

    W1hi_f = _q8(W1 * t1).astype(np.float32)
    W1hi = _q8(kmajor(W1hi_f))
    W1lo = _q8(kmajor(W1 * t1 - W1hi_f))
    W2q = _q8(kmajor(W2 * t2))
    w3s = (W3[:, 0] / (s1 * t2)).reshape(KB, 128).T.copy()

    def bias_lhsT(b, S):
        bhi = _q8(b * S / 32.0).astype(np.float32)
        blo = _q8((b * S - 32.0 * bhi) / 32.0).astype(np.float32)
        L = np.zeros((128, 2, HID), np.float32)
        L[:32, 0, :] = bhi[None, :]
        L[32:64, 0, :] = blo[None, :]
        return L.astype(E4)

    shared = {
        "yT": (y * s0).T.copy(),
        "Wx": Wx,
        "Wy": Wy,
        "b0s": ((b0 * s0).reshape(KB, 128).T).copy(),
        "W1hi": W1hi,
        "W1lo": W1lo,
        "W2q": W2q,
        "w3s": w3s.astype(np.float32),
        "onebf": np.ones((128, 1), np.float32).astype(BF),
        "bias1L": bias_lhsT(b1, s0 * t1),
        "bias2L": bias_lhsT(b2, s1 * t2),
        "ones8": np.ones((128, 2, PAIR), np.float32).astype(E4),
    }
    in_maps = []
    for c in range(N_CORES):
        im = dict(shared)
        im["xT"] = ((x[c * ROWS : (c + 1) * ROWS] * s0).T).copy()
        in_maps.append(im)
    scales = (float(s1 / (s0 * t1)),)
    return scales, in_maps


def run(trace=False, **inputs):
    scales, in_maps = _prep(inputs)
    if _CACHE.get("scales") != scales:
        _CACHE["nc"] = _build_nc(*scales)
        _CACHE["scales"] = scales
    nc = _CACHE["nc"]
    res = run_bass_kernel_spmd(nc, in_maps, core_ids=list(range(N_CORES)), trace=trace)
    b3 = np.float32(np.asarray(inputs["b3"]).reshape(-1)[0])
    blocks = [r["out"].reshape(ROWS, B) + b3 for r in res.results]
    return np.concatenate(blocks, axis=0).astype(np.float32), res


def _get_nc():
    return _CACHE["nc"]


def kernel(**inputs):
    out, _ = run(trace=False, **inputs)
    return out
